# revision 1
# baseline (speedup 1.0000x reference)
"""Trainium2 Bass kernel: LinkDecoder GNN edge MLP.

y[e] = relu(concat(x[src[e]], x[dst[e]]) @ W1 + b1) @ W2 + b2   (DOUT=1)

Strategy (8 NeuronCores, pure data-parallel over edges):
  * Host folds |W2| into W1 (relu(a)*w == sign(w)*relu(a*|w|)), casts x/W1 to
    fp16, and buckets edges by (src//25000, dst//25000) into 16 buckets,
    balanced across the 8 cores so every core runs the IDENTICAL static
    schedule: 128 tiles x 512 edges, tile t reads x-quarter pair
    (t//8 // 4, t//8 % 4).  Local row indices then fit in int16, which
    unlocks dma_gather(transpose=True): gathered node rows land
    FEATURE-major in SBUF ([128 part, 2 chunks, 512 edges]) - exactly the
    matmul rhs layout - with no on-chip transpose.
  * L1: 8 matmuls (4 k-chunks x 2 m-chunks) N=512 per tile, W1 tiles
    stationary in SBUF.  ACT applies relu(+b1*|W2|) PSUM->SBUF fp16.
  * L2: 2 matmuls lhsT=sign(W2) chunk [128,1] accumulate to psum [1,512];
    DVE adds b2 and casts to f32; HWDGE DMA to DRAM.
  * Host scatters per-core outputs back to original edge order.
"""

import numpy as np

N_NODES = 100000
DIN = 256
E_EDGES = 500000
NCORES = 8
NQ = 4                   # x row-quarters so local gather idx fits int16
QROWS = N_NODES // NQ    # 25000
TILE = 512               # edges per tile
TPB = 8                  # tiles per bucket (per core)
NBUCKET = NQ * NQ        # 16
NTILES = NBUCKET * TPB   # 128
CAP = TPB * TILE         # 4096 edges per bucket per core
EPC = NTILES * TILE      # 65536 padded edges per core

_CACHE = {}
LAST_RESULTS = None      # BassKernelResults of the most recent run (for test.py)


def _build_nc():
    import concourse.bacc as bacc
    import concourse.mybir as mybir
    import concourse.tile as tile

    f16 = mybir.dt.float16
    f32 = mybir.dt.float32
    i16 = mybir.dt.int16
    Relu = mybir.ActivationFunctionType.Relu

    nc = bacc.Bacc("TRN2", target_bir_lowering=False, debug=False,
                   num_devices=NCORES)

    xh = nc.dram_tensor("xh", [N_NODES, DIN], f16, kind="ExternalInput").ap()
    w1 = nc.dram_tensor("w1", [128, 8 * 128], f16, kind="ExternalInput").ap()
    sv = nc.dram_tensor("svec", [128, 2], f16, kind="ExternalInput").ap()
    b1v = nc.dram_tensor("b1v", [128, 2], f32, kind="ExternalInput").ap()
    b2v = nc.dram_tensor("b2v", [1, 1], f32, kind="ExternalInput").ap()
    idx = nc.dram_tensor("idx", [128, NTILES * 2 * (TILE // 16)], i16,
                         kind="ExternalInput").ap()
    y = nc.dram_tensor("y", [NTILES, TILE], f32, kind="ExternalOutput").ap()

    IDXW = TILE // 16    # idx columns per (tile, endpoint) block

    with tile.TileContext(nc) as tc:
        with (
            tc.tile_pool(name="const", bufs=1) as cpool,
            tc.tile_pool(name="gather", bufs=6) as gpool,
            tc.tile_pool(name="hid", bufs=3) as hpool,
            tc.tile_pool(name="yout", bufs=3) as ypool,
            tc.tile_pool(name="psh", bufs=4, space="PSUM") as pph,
            tc.tile_pool(name="psy", bufs=2, space="PSUM") as ppy,
        ):
            w1_sb = cpool.tile([128, 8 * 128], f16)
            nc.sync.dma_start(w1_sb, w1)
            s_sb = cpool.tile([128, 2], f16)
            nc.sync.dma_start(s_sb, sv)
            b1_sb = cpool.tile([128, 2], f32)
            nc.sync.dma_start(b1_sb, b1v)
            b2_sb = cpool.tile([1, 1], f32)
            nc.sync.dma_start(b2_sb, b2v)
            idx_sb = cpool.tile([128, NTILES * 2 * IDXW], i16)
            nc.sync.dma_start(idx_sb, idx)

            for t in range(NTILES):
                b = t // TPB
                sq, dq = b // NQ, b % NQ
                xi = gpool.tile([128, 2, TILE], f16, tag="g")
                xj = gpool.tile([128, 2, TILE], f16, tag="g")
                nc.gpsimd.dma_gather(
                    xi[:], xh[sq * QROWS:(sq + 1) * QROWS, :],
                    idx_sb[:, (2 * t) * IDXW:(2 * t + 1) * IDXW],
                    num_idxs=TILE, num_idxs_reg=TILE, elem_size=DIN,
                    transpose=True)
                nc.gpsimd.dma_gather(
                    xj[:], xh[dq * QROWS:(dq + 1) * QROWS, :],
                    idx_sb[:, (2 * t + 1) * IDXW:(2 * t + 2) * IDXW],
                    num_idxs=TILE, num_idxs_reg=TILE, elem_size=DIN,
                    transpose=True)

                h2 = hpool.tile([128, 2, TILE], f16, tag="h2")
                for m in range(2):
                    h_ps = pph.tile([128, TILE], f32, tag="h")
                    for kc in range(4):
                        rhs = (xi if kc < 2 else xj)[:, kc % 2, :]
                        nc.tensor.matmul(
                            h_ps, w1_sb[:, (kc * 2 + m) * 128:(kc * 2 + m + 1) * 128],
                            rhs, start=(kc == 0), stop=(kc == 3))
                    nc.scalar.activation(h2[:, m, :], h_ps, Relu,
                                         bias=b1_sb[:, m:m + 1])

                y_ps = ppy.tile([1, TILE], f32, tag="y")
                for m in range(2):
                    nc.tensor.matmul(y_ps, s_sb[:, m:m + 1], h2[:, m, :],
                                     start=(m == 0), stop=(m == 1))
                y_sb = ypool.tile([1, TILE], f32, tag="ysb")
                nc.vector.tensor_scalar_add(y_sb, y_ps, b2_sb)
                nc.sync.dma_start(y[t:t + 1, :], y_sb)

    nc.compile()
    return nc


def _prep_inputs(x, edge_label_index, W1, b1, W2, b2):
    """Host-side staging: fold W2, cast fp16, bucket+balance edges."""
    x16 = np.asarray(x, dtype=np.float32).astype(np.float16)
    W1 = np.asarray(W1, dtype=np.float32)
    W2 = np.asarray(W2, dtype=np.float32)
    b1 = np.asarray(b1, dtype=np.float32)
    b2 = np.asarray(b2, dtype=np.float32)

    a2 = np.abs(W2[:, 0])                       # [256]
    W1p = (W1 * a2[None, :]).astype(np.float16)  # [512, 256]
    sgn = np.sign(W2[:, 0]).astype(np.float16)   # [256]
    b1p = (b1 * a2).astype(np.float32)           # [256]

    # W1 tiles: w1sb[p, (kc*2+m)*128 + mm] = W1p[kc*128+p, m*128+mm]
    w1sb = np.ascontiguousarray(
        W1p.reshape(4, 128, 2, 128).transpose(1, 0, 2, 3).reshape(128, 1024))
    ssb = np.ascontiguousarray(sgn.reshape(2, 128).T)          # [128, 2]
    b1sb = np.ascontiguousarray(b1p.reshape(2, 128).T)         # [128, 2]
    b2sb = b2.reshape(1, 1)

    eli = np.asarray(edge_label_index)
    src = eli[0].astype(np.int64)
    dst = eli[1].astype(np.int64)
    bkt = (src // QROWS) * NQ + (dst // QROWS)
    order = np.argsort(bkt, kind="stable")
    counts = np.bincount(bkt, minlength=NBUCKET)
    offs = np.concatenate([[0], np.cumsum(counts)])

    src_loc = np.zeros((NCORES, NBUCKET, CAP), np.int16)
    dst_loc = np.zeros((NCORES, NBUCKET, CAP), np.int16)
    pos = np.full((NCORES, NBUCKET, CAP), -1, np.int64)
    for b in range(NBUCKET):
        ids = order[offs[b]:offs[b + 1]]
        parts = np.array_split(ids, NCORES)
        for c, p in enumerate(parts):
            k = len(p)
            assert k <= CAP, f"bucket {b} core {c} overflow: {k} > {CAP}"
            pos[c, b, :k] = p
            src_loc[c, b, :k] = (src[p] - (b // NQ) * QROWS).astype(np.int16)
            dst_loc[c, b, :k] = (dst[p] - (b % NQ) * QROWS).astype(np.int16)

    # Wrap indices into the HW layout: idx j of a 512-list -> partition j%16,
    # column j//16; blocks ordered (tile, endpoint); replicated to 8x16 rows.
    idx_maps = []
    for c in range(NCORES):
        A = np.stack([src_loc[c].reshape(NTILES, TILE),
                      dst_loc[c].reshape(NTILES, TILE)], axis=1)  # [128,2,512]
        A = A.reshape(NTILES, 2, TILE // 16, 16)
        i16map = A.transpose(3, 0, 1, 2).reshape(16, -1)
        idx_maps.append(np.ascontiguousarray(np.tile(i16map, (8, 1))))

    in_maps = [{
        "xh": x16, "w1": w1sb, "svec": ssb, "b1v": b1sb, "b2v": b2sb,
        "idx": idx_maps[c],
    } for c in range(NCORES)]
    return in_maps, pos


def kernel(x, edge_label_index, W1, b1, W2, b2):
    global LAST_RESULTS
    import os
    from concourse.bass_utils import run_bass_kernel_spmd

    in_maps, pos = _prep_inputs(x, edge_label_index, W1, b1, W2, b2)

    if "nc" not in _CACHE:
        _CACHE["nc"] = _build_nc()
    nc = _CACHE["nc"]

    trace = bool(int(os.environ.get("KERNEL_TRACE", "0")))
    res = run_bass_kernel_spmd(nc, in_maps, core_ids=list(range(NCORES)),
                               trace=trace)
    LAST_RESULTS = res

    yfull = np.zeros((E_EDGES,), np.float32)
    for c in range(NCORES):
        p = pos[c].reshape(-1)
        m = p >= 0
        yfull[p[m]] = res.results[c]["y"].reshape(-1)[m]
    return yfull.reshape(E_EDGES, 1)


# revision 13
# speedup vs baseline: 3.9869x; 3.9869x over previous
"""Trainium2 Bass kernel: LinkDecoder GNN edge MLP.

y[e] = relu(concat(x[src[e]], x[dst[e]]) @ W1 + b1) @ W2 + b2   (DOUT=1)

Strategy (8 NeuronCores, pure data-parallel over edges):
  * Host folds |W2| into W1 (relu(a)*w == sign(w)*relu(a*|w|)), casts x/W1 to
    fp16, and buckets edges by (src//25000, dst//25000) into 16 buckets,
    balanced across the 8 cores so every core runs the IDENTICAL static
    schedule: 128 tiles x 512 edges, tile t reads x-quarter pair
    (t//8 // 4, t//8 % 4).  Local row indices then fit in int16, which
    unlocks dma_gather(transpose=True): gathered node rows land
    FEATURE-major in SBUF ([128 part, 2 chunks, 512 edges]) - exactly the
    matmul rhs layout - with no on-chip transpose.
  * L1: 8 matmuls (4 k-chunks x 2 m-chunks) N=512 per tile, W1 tiles
    stationary in SBUF.  ACT applies relu(+b1*|W2|) PSUM->SBUF fp16.
  * L2: 2 matmuls lhsT=sign(W2) chunk [128,1] accumulate to psum [1,512];
    DVE adds b2 and casts to f32; HWDGE DMA to DRAM.
  * Host scatters per-core outputs back to original edge order.
"""

import numpy as np

N_NODES = 100000
DIN = 256
E_EDGES = 500000
NCORES = 8
NQ = 4                   # x row-quarters so local gather idx fits int16
QROWS = N_NODES // NQ    # 25000
GTILE = 512              # edges per dma_gather (>512 crashes the ucode)
SUB = 512                # edges per matmul subtile (PSUM bank N limit)
NSUB = GTILE // SUB      # 4
GPB = 8                  # gather-tiles per bucket (per core)
NBUCKET = NQ * NQ        # 16
NGT = NBUCKET * GPB      # 32 gather-tiles per core
CAP = GPB * GTILE        # 4096 edges per bucket per core
NTILES = NGT * NSUB      # 128 output row-tiles of SUB edges
EPC = NGT * GTILE        # 65536 padded edges per core

_CACHE = {}
LAST_RESULTS = None      # BassKernelResults of the most recent run (for test.py)


def _build_nc(repeat=1):
    import concourse.bacc as bacc
    import concourse.mybir as mybir
    import concourse.tile as tile

    f16 = mybir.dt.float16
    f32 = mybir.dt.float32
    i16 = mybir.dt.int16
    Relu = mybir.ActivationFunctionType.Relu

    # dynamic_dma_scratch_size: SWDGE descriptor-ring carveout. A GTILE-idx
    # transpose gather emits ~GTILE m2s descriptors; the default 16 KiB ring
    # (1024 slots) overflows for GTILE=2048, so give it 4096 slots.
    nc = bacc.Bacc("TRN2", target_bir_lowering=False, debug=False,
                   num_devices=NCORES, dynamic_dma_scratch_size=65536,
                   num_swdge_queues=4)

    xh = nc.dram_tensor("xh", [N_NODES, DIN], f16, kind="ExternalInput").ap()
    w1 = nc.dram_tensor("w1", [128, 8 * 128], f16, kind="ExternalInput").ap()
    sv = nc.dram_tensor("svec", [128, 2], f16, kind="ExternalInput").ap()
    b1v = nc.dram_tensor("b1v", [128, 2], f32, kind="ExternalInput").ap()
    b2v = nc.dram_tensor("b2v", [1, 1], f32, kind="ExternalInput").ap()
    idx = nc.dram_tensor("idx", [128, NGT * 2 * (GTILE // 16)], i16,
                         kind="ExternalInput").ap()
    y = nc.dram_tensor("y", [NTILES, SUB], f32, kind="ExternalOutput").ap()

    IDXW = GTILE // 16   # idx columns per (gather-tile, endpoint) block
    Add = mybir.AluOpType.add
    Max = mybir.AluOpType.max

    with tile.TileContext(nc) as tc:
        with (
            tc.tile_pool(name="const", bufs=1) as cpool,
            tc.tile_pool(name="gather", bufs=4) as gpool,
            tc.tile_pool(name="hid", bufs=6) as hpool,
            tc.tile_pool(name="yout", bufs=3) as ypool,
            tc.tile_pool(name="psh", bufs=4, space="PSUM") as pph,
            tc.tile_pool(name="psy", bufs=2, space="PSUM") as ppy,
        ):
            w1_sb = cpool.tile([128, 8 * 128], f16)
            nc.sync.dma_start(w1_sb, w1)
            s_sb = cpool.tile([128, 2], f16)
            nc.sync.dma_start(s_sb, sv)
            b1_sb = cpool.tile([128, 2], f32)
            nc.sync.dma_start(b1_sb, b1v)
            b2_sb = cpool.tile([1, 1], f32)
            nc.sync.dma_start(b2_sb, b2v)
            idx_sb = cpool.tile([128, NGT * 2 * IDXW], i16)
            nc.sync.dma_start(idx_sb, idx)

            for g in [gg for _ in range(repeat) for gg in range(NGT)]:
                b = g // GPB
                sq, dq = b // NQ, b % NQ
                xi = gpool.tile([128, 2, GTILE], f16, tag="g")
                xj = gpool.tile([128, 2, GTILE], f16, tag="g")
                nc.gpsimd.dma_gather(
                    xi[:], xh[sq * QROWS:(sq + 1) * QROWS, :],
                    idx_sb[:, (2 * g) * IDXW:(2 * g + 1) * IDXW],
                    num_idxs=GTILE, num_idxs_reg=GTILE, elem_size=DIN,
                    transpose=True, queue_num=(2 * g) % 4)
                nc.gpsimd.dma_gather(
                    xj[:], xh[dq * QROWS:(dq + 1) * QROWS, :],
                    idx_sb[:, (2 * g + 1) * IDXW:(2 * g + 2) * IDXW],
                    num_idxs=GTILE, num_idxs_reg=GTILE, elem_size=DIN,
                    transpose=True, queue_num=(2 * g + 1) % 4)

                ysbg = ypool.tile([128, SUB], f32, tag="ysb")
                for s in range(NSUB):
                    t = g * NSUB + s
                    e0 = s * SUB
                    h2 = hpool.tile([128, 2, SUB], f16, tag="h2")
                    for m in range(2):
                        h_ps = pph.tile([128, SUB], f32, tag="h")
                        for kc in range(4):
                            rhs = (xi if kc < 2 else xj)[:, kc % 2, e0:e0 + SUB]
                            nc.tensor.matmul(
                                h_ps,
                                w1_sb[:, (kc * 2 + m) * 128:(kc * 2 + m + 1) * 128],
                                rhs, start=(kc == 0), stop=(kc == 3))
                        # relu(x + b1): m=0 on ACT, m=1 on DVE (add+max fused)
                        if m == 0:
                            nc.scalar.activation(h2[:, m, :], h_ps, Relu,
                                                 bias=b1_sb[:, m:m + 1])
                        else:
                            nc.vector.tensor_scalar(
                                h2[:, m, :], h_ps, b1_sb[:, m:m + 1], 0.0,
                                op0=Add, op1=Max)

                    y_ps = ppy.tile([1, SUB], f32, tag="y")
                    for m in range(2):
                        nc.tensor.matmul(y_ps, s_sb[:, m:m + 1], h2[:, m, :],
                                         start=(m == 0), stop=(m == 1))
                    if s % 2 == 0:
                        nc.vector.tensor_scalar_add(
                            ysbg[32 * s:32 * s + 1, :], y_ps, b2_sb)
                    else:
                        nc.scalar.add(ysbg[32 * s:32 * s + 1, :], y_ps, b2_sb)
                ysbg_rows = ysbg.rearrange("(a b) n -> a b n", b=32)[:NSUB, 0, :]
                nc.sync.dma_start(y[g * NSUB:(g + 1) * NSUB, :], ysbg_rows)

    nc.compile()
    return nc


def _prep_inputs(x, edge_label_index, W1, b1, W2, b2):
    """Host-side staging: fold W2, cast fp16, bucket+balance edges."""
    x16 = np.asarray(x, dtype=np.float32).astype(np.float16)
    W1 = np.asarray(W1, dtype=np.float32)
    W2 = np.asarray(W2, dtype=np.float32)
    b1 = np.asarray(b1, dtype=np.float32)
    b2 = np.asarray(b2, dtype=np.float32)

    a2 = np.abs(W2[:, 0])                       # [256]
    W1p = (W1 * a2[None, :]).astype(np.float16)  # [512, 256]
    sgn = np.sign(W2[:, 0]).astype(np.float16)   # [256]
    b1p = (b1 * a2).astype(np.float32)           # [256]

    # W1 tiles: w1sb[p, (kc*2+m)*128 + mm] = W1p[kc*128+p, m*128+mm]
    w1sb = np.ascontiguousarray(
        W1p.reshape(4, 128, 2, 128).transpose(1, 0, 2, 3).reshape(128, 1024))
    ssb = np.ascontiguousarray(sgn.reshape(2, 128).T)          # [128, 2]
    b1sb = np.ascontiguousarray(b1p.reshape(2, 128).T)         # [128, 2]
    b2sb = b2.reshape(1, 1)

    eli = np.asarray(edge_label_index)
    src = eli[0].astype(np.int64)
    dst = eli[1].astype(np.int64)
    bkt = (src // QROWS) * NQ + (dst // QROWS)
    order = np.argsort(bkt, kind="stable")
    counts = np.bincount(bkt, minlength=NBUCKET)
    offs = np.concatenate([[0], np.cumsum(counts)])

    src_loc = np.zeros((NCORES, NBUCKET, CAP), np.int16)
    dst_loc = np.zeros((NCORES, NBUCKET, CAP), np.int16)
    pos = np.full((NCORES, NBUCKET, CAP), -1, np.int64)
    for b in range(NBUCKET):
        ids = order[offs[b]:offs[b + 1]]
        parts = np.array_split(ids, NCORES)
        for c, p in enumerate(parts):
            k = len(p)
            assert k <= CAP, f"bucket {b} core {c} overflow: {k} > {CAP}"
            pos[c, b, :k] = p
            src_loc[c, b, :k] = (src[p] - (b // NQ) * QROWS).astype(np.int16)
            dst_loc[c, b, :k] = (dst[p] - (b % NQ) * QROWS).astype(np.int16)

    # Wrap indices into the HW layout: idx j of a GTILE-list -> partition
    # j%16, column j//16; blocks ordered (gather-tile, endpoint); replicated
    # to the 8x16 partition rows.
    idx_maps = []
    for c in range(NCORES):
        A = np.stack([src_loc[c].reshape(NGT, GTILE),
                      dst_loc[c].reshape(NGT, GTILE)], axis=1)  # [NGT,2,GTILE]
        A = A.reshape(NGT, 2, GTILE // 16, 16)
        i16map = A.transpose(3, 0, 1, 2).reshape(16, -1)
        idx_maps.append(np.ascontiguousarray(np.tile(i16map, (8, 1))))

    in_maps = [{
        "xh": x16, "w1": w1sb, "svec": ssb, "b1v": b1sb, "b2v": b2sb,
        "idx": idx_maps[c],
    } for c in range(NCORES)]
    return in_maps, pos


def kernel(x, edge_label_index, W1, b1, W2, b2):
    global LAST_RESULTS
    import os
    from concourse.bass_utils import run_bass_kernel_spmd

    in_maps, pos = _prep_inputs(x, edge_label_index, W1, b1, W2, b2)

    if "nc" not in _CACHE:
        _CACHE["nc"] = _build_nc()
    nc = _CACHE["nc"]

    trace = bool(int(os.environ.get("KERNEL_TRACE", "0")))
    res = run_bass_kernel_spmd(nc, in_maps, core_ids=list(range(NCORES)),
                               trace=trace)
    LAST_RESULTS = res

    yfull = np.zeros((E_EDGES,), np.float32)
    for c in range(NCORES):
        p = pos[c].reshape(-1)
        m = p >= 0
        yfull[p[m]] = res.results[c]["y"].reshape(-1)[m]
    return yfull.reshape(E_EDGES, 1)
